# revision 1
# baseline (speedup 1.0000x reference)
"""Trainium2 Bass kernel for a (quirky) transformer decoder layer.

Problem shapes: B=2, S=2048, D=128, H=8 heads, head_dim=16.
  sa  = attn(q=x_tgt, kv=x_tgt);  r1 = sa @ w1 + b1 + x_tgt
  ca  = attn(q=enc_out, kv=x_tgt); r2 = ca @ w2 + b2 + r1
  ln  = (r2 - mean) / var   (var unbiased, divide by var not std)
  out = relu(ln @ w3 + b3) @ w4 + b4 + r2
(mask_src / mask_tgt are unused by the reference.)

Sharding: 8 cores, query-row sharding (zero communication). Core c handles
batch c//4, query rows [(c%4)*512 : (c%4+1)*512]. K/V are computed per-core
from the full 2048-row x_tgt of its batch (small replicated work).

On-chip layout: activations kept transposed [d, q] so weights are stationary
matmul operands. Scores are computed transposed (s^T[k, q]) via the fused
projection G_h = wk_h^T @ Q_h^T so that s^T = x @ G_h contracts over the full
128 input channels. Softmax skips max-subtraction (scores bounded ~|3| after
the 1/4 scale), exp runs on ScalarE reading 4 PSUM banks per instruction
(free dim 2048 = 4 heads x 512 queries), the denominator comes from an
all-ones column in a head-packed V (pv writes 4 heads into one PSUM bank via
32-column tile_position groups), and normalization broadcasts the reciprocal
denominator across partitions with a selector matmul.
"""

import numpy as np

import concourse.bass as bass
import concourse.tile as tile
from concourse import mybir
from concourse.bass_utils import run_bass_kernel_spmd

B, S, D, H, HD = 2, 2048, 128, 8, 16
QC = 512  # query rows per core
NCORES = 8
KT = 16  # number of 128-row key tiles
F32 = mybir.dt.float32
F32R = mybir.dt.float32r
AF = mybir.ActivationFunctionType
OP = mybir.AluOpType


# ---------------------------------------------------------------- host packing
def _pack32_cols(w, grp):
    """[D, 128]: col 32g+j (j<16) = w[:, j*H + (4*grp+g)], else 0.

    Used for wq (query projection producing 32-row-padded Q^T) and wv
    (value projection producing the head-packed V; col 32g+16 stays 0 and is
    later memset to 1 on device for the softmax denominator)."""
    out = np.zeros((D, 128), np.float32)
    for g in range(4):
        h = 4 * grp + g
        for j in range(HD):
            out[:, 32 * g + j] = w[:, j * H + h]
    return out


def _perm_head_major(w):
    """[D, D]: col 16h+j = w[:, j*H+h] (head-major column order)."""
    idx = np.empty(D, np.int64)
    for h in range(H):
        for j in range(HD):
            idx[16 * h + j] = j * H + h
    return np.ascontiguousarray(w[:, idx])


def _wk_head_T(w):
    """[16, H*D]: row j, cols 128h:128h+128 = wk[:, j*H+h]."""
    out = np.zeros((HD, H * D), np.float32)
    for h in range(H):
        for j in range(HD):
            out[j, 128 * h : 128 * (h + 1)] = w[:, j * H + h]
    return out


def _pack_w12(w, grp):
    """lhsT for the merge projection: row 32c+j = w[j*H + (4*grp+c), :]."""
    out = np.zeros((D, D), np.float32)
    for c in range(4):
        h = 4 * grp + c
        for j in range(HD):
            out[32 * c + j, :] = w[j * H + h, :]
    return out


def _shuf(a):
    """[T*128, 128] -> [128, T*128]: natural 128-row tiles along free dim."""
    t = a.shape[0] // 128
    return np.ascontiguousarray(
        a.reshape(t, 128, 128).transpose(1, 0, 2).reshape(128, t * 128)
    )


def _unshuf(y):
    """[128, 512] -> [512, 128]"""
    return y.reshape(128, 4, 128).transpose(1, 0, 2).reshape(512, 128)


def _sel_matrix():
    sel = np.zeros((128, 128), np.float32)
    for m in range(128):
        sel[32 * (m // 32) + 16, m] = 1.0
    return sel


def _split_multiwaits(nc):
    """Post-pass for walrus builds that accept only ONE sync-wait per
    instruction: split every instruction carrying N>1 waits into (N-1)
    single-wait NOPs on the same engine placed immediately before it."""
    uid = 0
    for f in nc.m.functions:
        for bb in f.blocks:
            il = bb.instructions
            if not any(
                i.sync_info is not None
                and i.sync_info.on_wait
                and len(i.sync_info.on_wait) > 1
                for i in il
            ):
                continue
            out = []
            for inst in il:
                si = inst.sync_info
                if si is not None and si.on_wait and len(si.on_wait) > 1:
                    waits = list(si.on_wait)
                    for w in waits[:-1]:
                        uid += 1
                        nop = mybir.InstNoOp(
                            name=f"WSPLIT-{uid}",
                            engine=inst.engine,
                            ins=[],
                            outs=[],
                            sync_info=mybir.SyncInfo(on_wait=[w], on_update=[]),
                        )
                        out.append(nop)
                    inst.sync_info = mybir.SyncInfo(
                        on_wait=[waits[-1]], on_update=list(si.on_update)
                    )
                out.append(inst)
            bb.instructions = out
    return nc


# ---------------------------------------------------------------- device build
def build_nc():
    nc = bass.Bass()

    def din(name, shape, dt=F32):
        return nc.dram_tensor(name, list(shape), dt, kind="ExternalInput")

    xb = din("xb", (128, 2048))  # batch x_tgt, 128-row tiles along free dim
    xq = din("xq", (128, 512))  # this core's x_tgt query slice
    eo = din("eo", (128, 512))  # this core's enc_out query slice
    wqh = [din(f"wqh{a}", (D, D), F32R) for a in range(2)]  # head-major cols
    wkh = [din(f"wkh{a}", (HD, H * D), F32R) for a in range(2)]  # wk_h^T stack
    wv_st = din("wv_st", (D, 512), F32R)  # [v_selfA | v_selfB | v_crossA | v_crossB]
    w1p = [din(f"w1p{g}", (D, D), F32R) for g in range(2)]
    w2p = [din(f"w2p{g}", (D, D), F32R) for g in range(2)]
    w3 = din("w3", (D, 512), F32R)
    w4r = din("w4r", (128, 512), F32R)  # col block j = w4[128j:128j+128, :]
    ones_v = din("ones_v", (128, 256), F32R)  # V-aug denominator columns
    selt = din("selt", (128, 128))  # SEL[p, m] = (p == 32*(m//32)+16)
    ident = din("ident", (128, 128))
    ones_col = din("ones_col", (128, 1))
    ones_row = din("ones_row", (1, 128))
    b1t = din("b1t", (128, 1))
    b2t = din("b2t", (128, 1))
    b3t = din("b3t", (128, 4))
    b4t = din("b4t", (128, 1))
    y = nc.dram_tensor("y", [128, 512], F32, kind="ExternalOutput")

    with tile.TileContext(nc) as tc:
        with tc.tile_pool(name="persist", bufs=1) as pp:

            def sbuf(name, shape, dt=F32):
                return pp.tile(list(shape), dt, name=name, tag=name)

            def load(name, dram, shape, dt=F32):
                t = sbuf(name, shape, dt)
                nc.sync.dma_start(out=t[:], in_=dram[:])
                return t

            # ---- constant / weight loads
            wq_t = [load(f"wq{a}", wqh[a], (D, D), F32R) for a in range(2)]
            wk_t = [load(f"wk{a}", wkh[a], (HD, H * D), F32R) for a in range(2)]
            wv_t = load("wv", wv_st, (D, 512), F32R)
            w1p_t = [load(f"w1p{g}", w1p[g], (D, D), F32R) for g in range(2)]
            w2p_t = [load(f"w2p{g}", w2p[g], (D, D), F32R) for g in range(2)]
            w3_t = load("w3", w3, (D, 512), F32R)
            w4_t = load("w4", w4r, (128, 512), F32R)
            sel_t = load("sel", selt, (128, 128))
            id_t = load("id", ident, (128, 128))
            onec_t = load("onec", ones_col, (128, 1))
            oner_t = load("oner", ones_row, (1, 128))
            b1_t = load("b1", b1t, (128, 1))
            b2_t = load("b2", b2t, (128, 1))
            b3_t = load("b3", b3t, (128, 4))
            b4_t = load("b4", b4t, (128, 1))

            xb_t = load("xbn", xb, (128, 2048))
            xq_t = load("xqn", xq, (128, 512))
            eo_t = load("eon", eo, (128, 512))

            xbT = sbuf("xbT", (128, 2048), F32R)
            xqT = sbuf("xqT", (128, 512), F32R)
            eoT = sbuf("eoT", (128, 512), F32R)
            v_all = sbuf("v_all", (128, 16, 512), F32R)
            g_s = [sbuf(f"gs{h}", (128, 512), F32R) for h in range(H)]
            g_c = [sbuf(f"gc{h}", (128, 512), F32R) for h in range(H)]
            qth = [[sbuf(f"qh{a}{h}", (HD, 512), F32R) for h in range(H)]
                   for a in range(2)]

            # ---------------- setup phase: transposes + projections
            with tc.tile_pool(name="pset", bufs=2, space="PSUM") as pset:

                def transpose_into(dst_ap, src_ap, name):
                    ps = pset.tile([128, 128], F32, name=name, tag="tps")
                    nc.tensor.transpose(ps[:], src_ap, id_t[:])
                    nc.vector.tensor_copy(out=dst_ap, in_=ps[:])

                for j in range(4):
                    transpose_into(xqT[:, 128 * j : 128 * (j + 1)],
                                   xq_t[:, 128 * j : 128 * (j + 1)], f"trq{j}")
                for j in range(4):
                    transpose_into(eoT[:, 128 * j : 128 * (j + 1)],
                                   eo_t[:, 128 * j : 128 * (j + 1)], f"tre{j}")
                for j in range(16):
                    transpose_into(xbT[:, 128 * j : 128 * (j + 1)],
                                   xb_t[:, 128 * j : 128 * (j + 1)], f"trb{j}")

                # per-head Q^T: out [16, 512] = wq_hm[:, 16h:16h+16].T @ x^T
                for a, xsrc in ((0, xqT), (1, eoT)):
                    for h in range(H):
                        qps = pset.tile([HD, 512], F32, name=f"qps{a}{h}",
                                        tag="qps")
                        nc.tensor.matmul(
                            qps[:], lhsT=wq_t[a][:, 16 * h : 16 * (h + 1)],
                            rhs=xsrc[:], start=True, stop=True)
                        nc.scalar.copy(out=qth[a][h][:], in_=qps[:])

                # G_h = wk_h^T @ Q_h^T  (K=16 contraction at base partition 0)
                for a in range(2):
                    heads = g_s if a == 0 else g_c
                    for h in range(H):
                        gp = pset.tile([128, 512], F32, name=f"gp{a}{h}",
                                       tag="gps")
                        nc.tensor.matmul(
                            gp[:],
                            lhsT=wk_t[a][:, 128 * h : 128 * (h + 1)],
                            rhs=qth[a][h][:],
                            start=True, stop=True,
                        )
                        nc.scalar.copy(out=heads[h][:], in_=gp[:])

                # V_aug packed: x @ [wv_packs for all 4 (attn, grp)] per k-tile
                for t in range(KT):
                    vp = pset.tile([128, 512], F32, name=f"vp{t}", tag="vps")
                    nc.tensor.matmul(
                        vp[:],
                        lhsT=xbT[:, 128 * t : 128 * (t + 1)],
                        rhs=wv_t[:],
                        start=True, stop=True,
                    )
                    nc.vector.tensor_copy(out=v_all[:, t, :], in_=vp[:])
                # ones columns for the softmax-denominator rows
                nc.sync.dma_start(
                    out=v_all[:].rearrange("p t (c x) -> p t c x", x=32)[:, :, :, 16],
                    in_=ones_v[:].rearrange("p (t c) -> p t c", c=16),
                )

            # ---------------- attention loops
            def attention(ai, g_heads, acc_tag, pa):
                """Process 4 sets of 2 heads; each set runs all 16 k-tiles
                with a double-buffered [128, 1024] score tile (2 banks) and a
                per-head PSUM accumulator bank; accumulators drain into the
                packed [128, 512] layout via 32-aligned DVE copies."""
                packed = [pp.tile([128, 512], F32, name=f"acc{ai}{g}",
                                  tag=f"{acc_tag}{g}") for g in range(2)]
                with tc.tile_pool(name=f"ebp{ai}", bufs=3) as ebp:
                    for st in range(4):
                        h0 = 2 * st
                        pv = [pa.tile([32, 512], F32, name=f"pv{ai}{st}{i}",
                                      tag=f"pv{i}") for i in range(2)]
                        for t in range(KT):
                            sc = pa.tile([128, 1024], F32, bufs=2,
                                         name=f"sc{ai}{st}{t}", tag="sc")
                            for i in range(2):
                                nc.tensor.matmul(
                                    sc[:, 512 * i : 512 * (i + 1)],
                                    lhsT=xbT[:, 128 * t : 128 * (t + 1)],
                                    rhs=g_heads[h0 + i][:],
                                    start=True, stop=True,
                                )
                            eb = ebp.tile([128, 1024], F32R, name="eb",
                                          tag="eb")
                            nc.scalar.activation(eb[:], sc[:], AF.Exp,
                                                 scale=0.25)
                            for i in range(2):
                                h = h0 + i
                                v0 = 256 * ai + 128 * (h // 4) + 32 * (h % 4)
                                nc.tensor.matmul(
                                    pv[i][:],
                                    lhsT=v_all[:, t, v0 : v0 + 32],
                                    rhs=eb[:, 512 * i : 512 * (i + 1)],
                                    start=(t == 0), stop=(t == KT - 1),
                                    skip_group_check=True,
                                )
                        # drain the two head accumulators into packed layout
                        for i in range(2):
                            h = h0 + i
                            nc.vector.tensor_copy(
                                out=packed[h // 4][32 * (h % 4) : 32 * (h % 4) + 32, :],
                                in_=pv[i][:],
                            )
                return packed

            def normalize_and_project(ai, accs, wp_t, pa):
                sa_n = []
                for grp in range(2):
                    sbc = pa.tile([128, 512], F32, name=f"sbc{ai}{grp}",
                                  tag=f"ps{grp}")
                    nc.tensor.matmul(sbc[:], lhsT=sel_t[:], rhs=accs[grp][:],
                                     start=True, stop=True)
                    rb = pp.tile([128, 512], F32, name=f"rb{ai}{grp}",
                                 tag=f"rb{grp}")
                    nc.vector.reciprocal(out=rb[:], in_=sbc[:])
                    sn = pp.tile([128, 512], F32R, name=f"sn{ai}{grp}",
                                 tag=f"sn{grp}")
                    nc.vector.tensor_mul(sn[:], accs[grp][:], rb[:])
                    sa_n.append(sn)
                rp = pa.tile([128, 512], F32, name=f"rp{ai}", tag="ps0")
                for grp in range(2):
                    nc.tensor.matmul(rp[:], lhsT=wp_t[grp][:],
                                     rhs=sa_n[grp][:],
                                     start=(grp == 0), stop=(grp == 1))
                return rp

            with tc.tile_pool(name="pattn", bufs=1, space="PSUM") as pa:
                acc_s = attention(0, g_s, "acs", pa)
                rp1 = normalize_and_project(0, acc_s, w1p_t, pa)
                r1T = sbuf("r1T", (128, 512))
                nc.vector.tensor_add(r1T[:], rp1[:], xqT[:])
                nc.vector.tensor_scalar_add(r1T[:], r1T[:], b1_t[:])

                acc_c = attention(1, g_c, "acc", pa)
                rp2 = normalize_and_project(1, acc_c, w2p_t, pa)
                r2T = sbuf("r2T", (128, 512))
                nc.vector.tensor_add(r2T[:], rp2[:], r1T[:])
                nc.vector.tensor_scalar_add(r2T[:], r2T[:], b2_t[:])

            # ---------------- layernorm (x - m) / var, var unbiased
            with tc.tile_pool(name="ptail", bufs=1, space="PSUM") as pt:
                sq = sbuf("sq", (128, 512))
                nc.vector.tensor_mul(sq[:], r2T[:], r2T[:])
                mp = pt.tile([1, 512], F32, name="mp", tag="st0")
                nc.tensor.matmul(mp[:], lhsT=onec_t[:], rhs=r2T[:],
                                 start=True, stop=True)
                sp = pt.tile([1, 512], F32, name="sp", tag="st1")
                nc.tensor.matmul(sp[:], lhsT=onec_t[:], rhs=sq[:],
                                 start=True, stop=True)
                msb = sbuf("msb", (1, 512))
                nc.vector.tensor_copy(out=msb[:], in_=mp[:])
                ssb = sbuf("ssb", (1, 512))
                nc.vector.tensor_copy(out=ssb[:], in_=sp[:])
                t0 = sbuf("t0", (1, 512))
                nc.vector.tensor_mul(t0[:], msb[:], msb[:])
                nc.vector.tensor_scalar_mul(t0[:], t0[:], 1.0 / 128)
                nc.vector.tensor_sub(t0[:], ssb[:], t0[:])  # sum((x-m)^2)
                asb = sbuf("asb", (1, 512))
                nc.vector.reciprocal(out=asb[:], in_=t0[:])
                nc.vector.tensor_scalar_mul(asb[:], asb[:], 127.0)  # a = 1/var
                bsb = sbuf("bsb", (1, 512))
                nc.vector.tensor_mul(bsb[:], msb[:], asb[:])
                nc.vector.tensor_scalar_mul(bsb[:], bsb[:], -1.0 / 128)  # -m/var
                abc = pt.tile([128, 512], F32, name="abc", tag="bc0")
                nc.tensor.matmul(abc[:], lhsT=oner_t[:], rhs=asb[:],
                                 start=True, stop=True)
                bbc = pt.tile([128, 512], F32, name="bbc", tag="bc1")
                nc.tensor.matmul(bbc[:], lhsT=oner_t[:], rhs=bsb[:],
                                 start=True, stop=True)
                lnT = sbuf("lnT", (128, 512), F32R)
                nc.vector.tensor_mul(lnT[:], r2T[:], abc[:])
                nc.vector.tensor_add(lnT[:], lnT[:], bbc[:])

                # ---------------- FFN
                h_sb = []
                for j in range(4):
                    hp = pt.tile([128, 512], F32, name=f"hp{j}", tag=f"hp{j % 2}")
                    nc.tensor.matmul(hp[:],
                                     lhsT=w3_t[:, 128 * j : 128 * (j + 1)],
                                     rhs=lnT[:], start=True, stop=True)
                    hs = sbuf(f"hs{j}", (128, 512), F32R)
                    nc.vector.tensor_scalar(
                        out=hs[:], in0=hp[:], scalar1=b3_t[:, j : j + 1],
                        scalar2=0.0, op0=OP.add, op1=OP.max,
                    )
                    h_sb.append(hs)
                op_ = pt.tile([128, 512], F32, name="op", tag="bc0")
                for j in range(4):
                    nc.tensor.matmul(op_[:],
                                     lhsT=w4_t[:, 128 * j : 128 * (j + 1)],
                                     rhs=h_sb[j][:],
                                     start=(j == 0), stop=(j == 3),
                                     skip_group_check=True)
                oT = sbuf("oT", (128, 512))
                nc.vector.tensor_add(oT[:], op_[:], r2T[:])
                nc.vector.tensor_scalar_add(oT[:], oT[:], b4_t[:])

                # ------------- transpose back to natural rows and store
                on = sbuf("on", (128, 512))
                for j in range(4):
                    tp = pt.tile([128, 128], F32, name=f"tp{j}", tag=f"st{j % 2}")
                    nc.tensor.transpose(tp[:], oT[:, 128 * j : 128 * (j + 1)],
                                        id_t[:])
                    nc.vector.tensor_copy(out=on[:, 128 * j : 128 * (j + 1)],
                                          in_=tp[:])
                nc.sync.dma_start(out=y[:], in_=on[:])

    return nc


_CACHED = {}


def _get_nc():
    if "nc" not in _CACHED:
        _CACHED["nc"] = _split_multiwaits(build_nc())
    return _CACHED["nc"]


def _host_inputs(x_tgt, enc_out, self_wq, self_wk, self_wv, cross_wq, cross_wk,
                 cross_wv, w1, b1, w2, b2, w3, b3, w4, b4):
    shared = {
        "wqh0": _perm_head_major(self_wq), "wqh1": _perm_head_major(cross_wq),
        "wkh0": _wk_head_T(self_wk), "wkh1": _wk_head_T(cross_wk),
        "wv_st": np.concatenate(
            [_pack32_cols(self_wv, 0), _pack32_cols(self_wv, 1),
             _pack32_cols(cross_wv, 0), _pack32_cols(cross_wv, 1)], axis=1
        ),
        "w1p0": _pack_w12(w1, 0), "w1p1": _pack_w12(w1, 1),
        "w2p0": _pack_w12(w2, 0), "w2p1": _pack_w12(w2, 1),
        "w3": w3,
        "w4r": np.ascontiguousarray(
            w4.reshape(4, 128, 128).transpose(1, 0, 2).reshape(128, 512)
        ),
        "ones_v": np.ones((128, 256), np.float32),
        "selt": _sel_matrix(),
        "ident": np.eye(128, dtype=np.float32),
        "ones_col": np.ones((128, 1), np.float32),
        "ones_row": np.ones((1, 128), np.float32),
        "b1t": b1.reshape(128, 1),
        "b2t": b2.reshape(128, 1),
        "b3t": np.ascontiguousarray(b3.reshape(4, 128).T),
        "b4t": b4.reshape(128, 1),
    }
    shared = {k: np.ascontiguousarray(v, dtype=np.float32)
              for k, v in shared.items()}
    in_maps = []
    for c in range(NCORES):
        b, qb = divmod(c, 4)
        q0 = qb * QC
        im = dict(shared)
        im["xb"] = _shuf(x_tgt[b])
        im["xq"] = _shuf(x_tgt[b, q0 : q0 + QC])
        im["eo"] = _shuf(enc_out[b, q0 : q0 + QC])
        in_maps.append(im)
    return in_maps


def run_on_device(in_maps, **kw):
    nc = _get_nc()
    return run_bass_kernel_spmd(nc, in_maps, list(range(NCORES)), **kw)


def kernel(x_tgt, enc_out, self_wq, self_wk, self_wv, cross_wq, cross_wk,
           cross_wv, w1, b1, w2, b2, w3, b3, w4, b4, mask_src=None,
           mask_tgt=None, **_unused):
    args = [x_tgt, enc_out, self_wq, self_wk, self_wv, cross_wq, cross_wk,
            cross_wv, w1, b1, w2, b2, w3, b3, w4, b4]
    args = [np.asarray(a, dtype=np.float32) for a in args]
    in_maps = _host_inputs(*args)
    res = run_on_device(in_maps)
    out = np.empty((B, S, D), np.float32)
    for c in range(NCORES):
        b, qb = divmod(c, 4)
        out[b, qb * QC : (qb + 1) * QC] = _unshuf(res.results[c]["y"])
    return out



# revision 7
# speedup vs baseline: 1.1028x; 1.1028x over previous
"""Trainium2 Bass kernel for a (quirky) transformer decoder layer.

Problem shapes: B=2, S=2048, D=128, H=8 heads, head_dim=16.
  sa  = attn(q=x_tgt, kv=x_tgt);  r1 = sa @ w1 + b1 + x_tgt
  ca  = attn(q=enc_out, kv=x_tgt); r2 = ca @ w2 + b2 + r1
  ln  = (r2 - mean) / var   (var unbiased, divide by var not std)
  out = relu(ln @ w3 + b3) @ w4 + b4 + r2
(mask_src / mask_tgt are unused by the reference.)

Sharding: 8 cores, query-row sharding (zero communication). Core c handles
batch c//4, query rows [(c%4)*512 : (c%4+1)*512]. K/V are computed per-core
from the full 2048-row x_tgt of its batch (small replicated work).

v2 (vs the fp32r baseline): the trace showed PE 97.9% busy with every matmul
in fp32 two-pass mode (~428ns per 512-row op). This version:
  - runs the whole matmul path in bf16 (1 cycle/row, FWL-eligible weights);
  - sends x^T / enc^T pre-transposed from host (kills 24 PE transposes);
  - fuses the Q->G two-step into one matmul via host-precomputed
    M_h^T = 0.25 * wq_h @ wk_h^T  (scores^T = x @ (M_h @ x_q^T));
  - keeps PSUM->SBUF copies on DVE (ScalarE does exp exclusively);
  - normalizes per 4-head group, interleaved into the next attention's
    pipeline so PE never stalls on the DVE reciprocal;
  - returns the output transposed; host un-transposes.
Residuals r1/r2 and softmax denominators stay fp32 (accuracy anchor).
"""

import ml_dtypes
import numpy as np

import concourse.bass as bass
import concourse.tile as tile
from concourse import mybir
from concourse.bass_utils import run_bass_kernel_spmd

B, S, D, H, HD = 2, 2048, 128, 8, 16
QC = 512  # query rows per core
NCORES = 8
KT = 16  # number of 128-row key tiles
F32 = mybir.dt.float32
BF16 = mybir.dt.bfloat16
AF = mybir.ActivationFunctionType
OP = mybir.AluOpType
NPBF = ml_dtypes.bfloat16


# ---------------------------------------------------------------- host packing
def _head_cols(h):
    return [j * H + h for j in range(HD)]


def _fuse_qk(wq, wk):
    """[128, H*128] bf16: col block h = M_h^T = 0.25 * wq_h @ wk_h^T."""
    out = np.empty((D, H * D), np.float32)
    for h in range(H):
        c = _head_cols(h)
        out[:, D * h : D * (h + 1)] = 0.25 * (wq[:, c] @ wk[:, c].T)
    return out


def _pack32_cols(w, grp):
    """[D, 128]: col 32g+j (j<16) = w[:, j*H + (4*grp+g)], else 0."""
    out = np.zeros((D, 128), np.float32)
    for g in range(4):
        h = 4 * grp + g
        for j in range(HD):
            out[:, 32 * g + j] = w[:, j * H + h]
    return out


def _pack_w12(w, grp):
    """lhsT for the merge projection: row 32c+j = w[j*H + (4*grp+c), :]."""
    out = np.zeros((D, D), np.float32)
    for c in range(4):
        h = 4 * grp + c
        for j in range(HD):
            out[32 * c + j, :] = w[j * H + h, :]
    return out


def _sel_matrix():
    sel = np.zeros((128, 128), np.float32)
    for m in range(128):
        sel[32 * (m // 32) + 16, m] = 1.0
    return sel


def _split_multiwaits(nc):
    """Post-pass for walrus builds that accept only ONE sync-wait per
    instruction: split every instruction carrying N>1 waits into (N-1)
    single-wait NOPs on the same engine placed immediately before it."""
    uid = 0
    for f in nc.m.functions:
        for bb in f.blocks:
            il = bb.instructions
            if not any(
                i.sync_info is not None
                and i.sync_info.on_wait
                and len(i.sync_info.on_wait) > 1
                for i in il
            ):
                continue
            out = []
            for inst in il:
                si = inst.sync_info
                if si is not None and si.on_wait and len(si.on_wait) > 1:
                    waits = list(si.on_wait)
                    for w in waits[:-1]:
                        uid += 1
                        nop = mybir.InstNoOp(
                            name=f"WSPLIT-{uid}",
                            engine=inst.engine,
                            ins=[],
                            outs=[],
                            sync_info=mybir.SyncInfo(on_wait=[w], on_update=[]),
                        )
                        out.append(nop)
                    inst.sync_info = mybir.SyncInfo(
                        on_wait=[waits[-1]], on_update=list(si.on_update)
                    )
                out.append(inst)
            bb.instructions = out
    return nc


# ---------------------------------------------------------------- device build
def build_nc():
    nc = bass.Bass()

    def din(name, shape, dt=BF16):
        return nc.dram_tensor(name, list(shape), dt, kind="ExternalInput")

    xbT = din("xbT", (128, 2048))  # batch x_tgt transposed [chan, key]
    xqT = din("xqT", (128, 512))  # query slice of x_tgt, transposed
    eoT = din("eoT", (128, 512))  # query slice of enc_out, transposed
    xqF = din("xqF", (128, 512), F32)  # fp32 copy for the residual
    mgs = din("mgs", (128, 1024))  # self-attn fused M_h^T blocks
    mgc = din("mgc", (128, 1024))  # cross-attn fused M_h^T blocks
    wv_st = din("wv_st", (D, 512))  # [v_selfA | v_selfB | v_crossA | v_crossB]
    w1p = [din(f"w1p{g}", (D, D)) for g in range(2)]
    w2p = [din(f"w2p{g}", (D, D)) for g in range(2)]
    w3 = din("w3", (D, 512))
    w4r = din("w4r", (128, 512))  # col block j = w4[128j:128j+128, :]
    ones_v = din("ones_v", (128, 256))  # V-aug denominator columns
    selt = din("selt", (128, 128), F32)  # SEL[p, m] = (p == 32*(m//32)+16)
    onec_b = din("onec_b", (128, 1))  # bf16 ones column (LN sums)
    oner_b = din("oner_b", (1, 128))  # bf16 ones row (LN broadcast)
    b1t = din("b1t", (128, 1), F32)
    b2t = din("b2t", (128, 1), F32)
    b3t = din("b3t", (128, 4), F32)
    b4t = din("b4t", (128, 1), F32)
    y = nc.dram_tensor("y", [128, 512], F32, kind="ExternalOutput")

    with tile.TileContext(nc) as tc:
        with tc.tile_pool(name="persist", bufs=1) as pp:

            def sbuf(name, shape, dt=F32):
                return pp.tile(list(shape), dt, name=name, tag=name)

            def load(name, dram, shape, dt=BF16):
                t = sbuf(name, shape, dt)
                nc.sync.dma_start(out=t[:], in_=dram[:])
                return t

            # ---- loads ordered by first use
            mgs_t = load("mgs", mgs, (128, 1024))
            xqT_t = load("xqT", xqT, (128, 512))
            xbT_t = load("xbT", xbT, (128, 2048))
            wv_t = load("wv", wv_st, (D, 512))
            mgc_t = load("mgc", mgc, (128, 1024))
            eoT_t = load("eoT", eoT, (128, 512))
            sel_t = load("sel", selt, (128, 128), F32)
            w1p_t = [load(f"w1p{g}", w1p[g], (D, D)) for g in range(2)]
            w2p_t = [load(f"w2p{g}", w2p[g], (D, D)) for g in range(2)]
            xqF_t = load("xqF", xqF, (128, 512), F32)
            b1_t = load("b1", b1t, (128, 1), F32)
            b2_t = load("b2", b2t, (128, 1), F32)
            onec_t = load("onec", onec_b, (128, 1))
            oner_t = load("oner", oner_b, (1, 128))
            w3_t = load("w3", w3, (D, 512))
            w4_t = load("w4", w4r, (128, 512))
            b3_t = load("b3", b3t, (128, 4), F32)
            b4_t = load("b4", b4t, (128, 1), F32)

            v_all = sbuf("v_all", (128, KT, 512), BF16)
            g_s = [sbuf(f"gs{h}", (128, 512), BF16) for h in range(H)]
            g_c = [sbuf(f"gc{h}", (128, 512), BF16) for h in range(H)]

            # ---------------- setup: G projections + packed V
            with tc.tile_pool(name="pset", bufs=2, space="PSUM") as pset:
                # G_h = M_h @ x_q^T : scores^T tile = x_tile @ G_h
                for a, (msrc, xsrc, heads) in enumerate(
                    ((mgs_t, xqT_t, g_s), (mgc_t, eoT_t, g_c))
                ):
                    for h in range(H):
                        gp = pset.tile([128, 512], F32, name=f"gp{a}{h}",
                                       tag="gps")
                        nc.tensor.matmul(
                            gp[:],
                            lhsT=msrc[:, 128 * h : 128 * (h + 1)],
                            rhs=xsrc[:],
                            start=True, stop=True,
                        )
                        nc.vector.tensor_copy(out=heads[h][:], in_=gp[:])
                    if a == 0:
                        # V_aug packed: x @ [wv for all 4 (attn, grp)] per tile
                        for t in range(KT):
                            vp = pset.tile([128, 512], F32, name=f"vp{t}",
                                           tag="vps")
                            nc.tensor.matmul(
                                vp[:],
                                lhsT=xbT_t[:, 128 * t : 128 * (t + 1)],
                                rhs=wv_t[:],
                                start=True, stop=True,
                            )
                            nc.vector.tensor_copy(out=v_all[:, t, :], in_=vp[:])
                # ones columns for the softmax-denominator rows
                nc.sync.dma_start(
                    out=v_all[:].rearrange("p t (c x) -> p t c x", x=32)[:, :, :, 16],
                    in_=ones_v[:].rearrange("p (t c) -> p t c", c=16),
                )

            # ---------------- attention + interleaved per-group normalize
            with tc.tile_pool(name="pattn", bufs=1, space="PSUM") as pa, \
                 tc.tile_pool(name="ebp", bufs=3) as ebp:

                packed = {}  # (ai, grp) -> SBUF f32 accumulator
                rp = {}  # ai -> PSUM merge accumulator

                def attn_set(ai, st, g_heads):
                    """2 heads x 16 k-tiles: scores -> exp -> PV accumulate,
                    then drain the two head accumulators into packed layout."""
                    h0 = 2 * st
                    pv = [pa.tile([32, 512], F32, name=f"pv{ai}{st}{i}",
                                  tag=f"pv{i}") for i in range(2)]
                    for t in range(KT):
                        sc = pa.tile([128, 1024], F32, bufs=2,
                                     name=f"sc{ai}{st}{t}", tag="sc")
                        for i in range(2):
                            nc.tensor.matmul(
                                sc[:, 512 * i : 512 * (i + 1)],
                                lhsT=xbT_t[:, 128 * t : 128 * (t + 1)],
                                rhs=g_heads[h0 + i][:],
                                start=True, stop=True,
                            )
                        eb = ebp.tile([128, 1024], BF16, name="eb", tag="eb")
                        nc.scalar.activation(eb[:], sc[:], AF.Exp)
                        for i in range(2):
                            h = h0 + i
                            v0 = 256 * ai + 128 * (h // 4) + 32 * (h % 4)
                            nc.tensor.matmul(
                                pv[i][:],
                                lhsT=v_all[:, t, v0 : v0 + 32],
                                rhs=eb[:, 512 * i : 512 * (i + 1)],
                                start=(t == 0), stop=(t == KT - 1),
                                skip_group_check=True,
                            )
                    for i in range(2):
                        h = h0 + i
                        grp = h // 4
                        if (ai, grp) not in packed:
                            packed[(ai, grp)] = pp.tile(
                                [128, 512], F32, name=f"acc{ai}{grp}",
                                tag=f"acc{ai}{grp}")
                        nc.vector.tensor_copy(
                            out=packed[(ai, grp)][32 * (h % 4) : 32 * (h % 4) + 32, :],
                            in_=pv[i][:],
                        )

                def norm_grp(ai, grp, wp_t):
                    """softmax-normalize one 4-head group and accumulate its
                    merge projection into rp[ai]."""
                    acc = packed[(ai, grp)]
                    sbc = pa.tile([128, 512], F32, name=f"sbc{ai}{grp}",
                                  tag="ps1")
                    nc.tensor.matmul(sbc[:], lhsT=sel_t[:], rhs=acc[:],
                                     start=True, stop=True)
                    rb = pp.tile([128, 512], F32, name=f"rb{ai}{grp}",
                                 tag="rb")
                    nc.vector.reciprocal(out=rb[:], in_=sbc[:])
                    sn = pp.tile([128, 512], BF16, name=f"sn{ai}{grp}",
                                 tag=f"sn{grp}")
                    nc.vector.tensor_mul(sn[:], acc[:], rb[:])
                    if ai not in rp:
                        rp[ai] = pa.tile([128, 512], F32, name=f"rp{ai}",
                                         tag="ps0")
                    nc.tensor.matmul(rp[ai][:], lhsT=wp_t[grp][:], rhs=sn[:],
                                     start=(grp == 0), stop=(grp == 1),
                                     skip_group_check=True)

                # self-attention, grp0 normalize folded between its sets
                attn_set(0, 0, g_s)
                attn_set(0, 1, g_s)
                norm_grp(0, 0, w1p_t)
                attn_set(0, 2, g_s)
                attn_set(0, 3, g_s)
                norm_grp(0, 1, w1p_t)
                r1T = sbuf("r1T", (128, 512))
                # r1 = (sa@w1 + b1) + x_tgt
                nc.vector.scalar_tensor_tensor(
                    out=r1T[:], in0=rp[0][:], scalar=b1_t[:], in1=xqF_t[:],
                    op0=OP.add, op1=OP.add)

                # cross-attention
                attn_set(1, 0, g_c)
                attn_set(1, 1, g_c)
                norm_grp(1, 0, w2p_t)
                attn_set(1, 2, g_c)
                attn_set(1, 3, g_c)
                norm_grp(1, 1, w2p_t)
                r2T = sbuf("r2T", (128, 512))
                nc.vector.scalar_tensor_tensor(
                    out=r2T[:], in0=rp[1][:], scalar=b2_t[:], in1=r1T[:],
                    op0=OP.add, op1=OP.add)

            # ---------------- layernorm (x - m) / var, var unbiased
            with tc.tile_pool(name="ptail", bufs=1, space="PSUM") as pt:
                # bf16 [r2 | r2^2] for the column-sum matmul
                rsq = sbuf("rsq", (128, 1024), BF16)
                nc.vector.tensor_copy(out=rsq[:, :512], in_=r2T[:])
                nc.vector.tensor_mul(rsq[:, 512:], r2T[:], r2T[:])
                sp = pt.tile([1, 1024], F32, name="sp", tag="st")
                for j in range(2):
                    nc.tensor.matmul(sp[:, 512 * j : 512 * (j + 1)],
                                     lhsT=onec_t[:],
                                     rhs=rsq[:, 512 * j : 512 * (j + 1)],
                                     start=True, stop=True)
                spb = sbuf("spb", (1, 1024))
                nc.vector.tensor_copy(out=spb[:], in_=sp[:])
                # per-column stats (free dim 512, partition dim 1)
                tm = sbuf("tm", (1, 512))
                nc.vector.tensor_mul(tm[:], spb[:, :512], spb[:, :512])
                nc.vector.tensor_scalar_mul(tm[:], tm[:], 1.0 / 128)
                nc.vector.tensor_sub(tm[:], spb[:, 512:], tm[:])  # sum((x-m)^2)
                ra = sbuf("ra", (1, 512))
                nc.vector.reciprocal(out=ra[:], in_=tm[:])
                nc.vector.tensor_scalar_mul(ra[:], ra[:], 127.0)  # a = 1/var
                rb2 = sbuf("rb2", (1, 512))
                nc.vector.tensor_mul(rb2[:], spb[:, :512], ra[:])
                nc.vector.tensor_scalar_mul(rb2[:], rb2[:], -1.0 / 128)  # -m/var
                ab_b = sbuf("ab_b", (1, 1024), BF16)
                nc.vector.tensor_copy(out=ab_b[:, :512], in_=ra[:])
                nc.vector.tensor_copy(out=ab_b[:, 512:], in_=rb2[:])
                abc = pt.tile([128, 1024], F32, name="abc", tag="bc")
                for j in range(2):
                    nc.tensor.matmul(abc[:, 512 * j : 512 * (j + 1)],
                                     lhsT=oner_t[:],
                                     rhs=ab_b[:, 512 * j : 512 * (j + 1)],
                                     start=True, stop=True)
                lnf = sbuf("lnf", (128, 512))
                nc.vector.tensor_mul(lnf[:], r2T[:], abc[:, :512])
                lnT = sbuf("lnT", (128, 512), BF16)
                nc.vector.tensor_add(lnT[:], lnf[:], abc[:, 512:])

                # ---------------- FFN
                h_sb = []
                for j in range(4):
                    hp = pt.tile([128, 512], F32, name=f"hp{j}", tag=f"hp{j % 2}")
                    nc.tensor.matmul(hp[:],
                                     lhsT=w3_t[:, 128 * j : 128 * (j + 1)],
                                     rhs=lnT[:], start=True, stop=True)
                    hs = sbuf(f"hs{j}", (128, 512), BF16)
                    nc.vector.tensor_scalar(
                        out=hs[:], in0=hp[:], scalar1=b3_t[:, j : j + 1],
                        scalar2=0.0, op0=OP.add, op1=OP.max,
                    )
                    h_sb.append(hs)
                op_ = pt.tile([128, 512], F32, name="op", tag="bc")
                for j in range(4):
                    nc.tensor.matmul(op_[:],
                                     lhsT=w4_t[:, 128 * j : 128 * (j + 1)],
                                     rhs=h_sb[j][:],
                                     start=(j == 0), stop=(j == 3),
                                     skip_group_check=True)
                oT = sbuf("oT", (128, 512))
                nc.vector.scalar_tensor_tensor(
                    out=oT[:], in0=op_[:], scalar=b4_t[:], in1=r2T[:],
                    op0=OP.add, op1=OP.add)
                nc.sync.dma_start(out=y[:], in_=oT[:])

    return nc


_CACHED = {}


def _get_nc():
    if "nc" not in _CACHED:
        _CACHED["nc"] = _split_multiwaits(build_nc())
    return _CACHED["nc"]


def _host_inputs(x_tgt, enc_out, self_wq, self_wk, self_wv, cross_wq, cross_wk,
                 cross_wv, w1, b1, w2, b2, w3, b3, w4, b4):
    bf = lambda a: np.ascontiguousarray(a, dtype=NPBF)
    f32 = lambda a: np.ascontiguousarray(a, dtype=np.float32)
    shared = {
        "mgs": bf(_fuse_qk(self_wq, self_wk)),
        "mgc": bf(_fuse_qk(cross_wq, cross_wk)),
        "wv_st": bf(np.concatenate(
            [_pack32_cols(self_wv, 0), _pack32_cols(self_wv, 1),
             _pack32_cols(cross_wv, 0), _pack32_cols(cross_wv, 1)], axis=1
        )),
        "w1p0": bf(_pack_w12(w1, 0)), "w1p1": bf(_pack_w12(w1, 1)),
        "w2p0": bf(_pack_w12(w2, 0)), "w2p1": bf(_pack_w12(w2, 1)),
        "w3": bf(w3),
        "w4r": bf(w4.reshape(4, 128, 128).transpose(1, 0, 2).reshape(128, 512)),
        "ones_v": bf(np.ones((128, 256), np.float32)),
        "selt": f32(_sel_matrix()),
        "onec_b": bf(np.ones((128, 1), np.float32)),
        "oner_b": bf(np.ones((1, 128), np.float32)),
        "b1t": f32(b1.reshape(128, 1)),
        "b2t": f32(b2.reshape(128, 1)),
        "b3t": f32(b3.reshape(4, 128).T),
        "b4t": f32(b4.reshape(128, 1)),
    }
    in_maps = []
    for c in range(NCORES):
        b, qb = divmod(c, 4)
        q0 = qb * QC
        im = dict(shared)
        im["xbT"] = bf(x_tgt[b].T)
        im["xqT"] = bf(x_tgt[b, q0 : q0 + QC].T)
        im["xqF"] = f32(x_tgt[b, q0 : q0 + QC].T)
        im["eoT"] = bf(enc_out[b, q0 : q0 + QC].T)
        in_maps.append(im)
    return in_maps


def _unshuf(y):
    """[128, 512] transposed output -> [512, 128] natural rows."""
    return np.ascontiguousarray(y.T)


def run_on_device(in_maps, **kw):
    nc = _get_nc()
    return run_bass_kernel_spmd(nc, in_maps, list(range(NCORES)), **kw)


def kernel(x_tgt, enc_out, self_wq, self_wk, self_wv, cross_wq, cross_wk,
           cross_wv, w1, b1, w2, b2, w3, b3, w4, b4, mask_src=None,
           mask_tgt=None, **_unused):
    args = [x_tgt, enc_out, self_wq, self_wk, self_wv, cross_wq, cross_wk,
            cross_wv, w1, b1, w2, b2, w3, b3, w4, b4]
    args = [np.asarray(a, dtype=np.float32) for a in args]
    in_maps = _host_inputs(*args)
    res = run_on_device(in_maps)
    out = np.empty((B, S, D), np.float32)
    for c in range(NCORES):
        b, qb = divmod(c, 4)
        out[b, qb * QC : (qb + 1) * QC] = _unshuf(res.results[c]["y"])
    return out


# revision 10
# speedup vs baseline: 1.2756x; 1.1568x over previous
"""Trainium2 Bass kernel for a (quirky) transformer decoder layer.

Problem shapes: B=2, S=2048, D=128, H=8 heads, head_dim=16.
  sa  = attn(q=x_tgt, kv=x_tgt);  r1 = sa @ w1 + b1 + x_tgt
  ca  = attn(q=enc_out, kv=x_tgt); r2 = ca @ w2 + b2 + r1
  ln  = (r2 - mean) / var   (var unbiased, divide by var not std)
  out = relu(ln @ w3 + b3) @ w4 + b4 + r2
(mask_src / mask_tgt are unused by the reference.)

Sharding: 8 cores, query-row sharding (zero communication). Core c handles
batch c//4, query rows [(c%4)*512 : (c%4+1)*512]. K/V are computed per-core
from the full 2048-row x_tgt of its batch (small replicated work).

The kernel is ScalarE-bound: softmax exp over 2 x 8 heads x 2048 x 512
scores = 16.8M elements/core = ~110us of ACT time; everything else hides
under it. Structure:
  - all matmuls bf16 (1 cycle/row + FWL weights); fp32 only for residuals,
    softmax denominators and layernorm stats;
  - x^T / enc^T arrive pre-transposed from host; Q->K fused on host into
    M_h^T = 0.25 * wq_h @ wk_h^T so scores^T = x @ (M_h @ x_q^T);
  - inner pipeline issues scores(t) ahead of PV(t-1) so the exp stream on
    ScalarE never waits on PE;
  - setup matmuls (G heads, packed V) are woven one-per-tile into the first
    attention sets, sharing a single PSUM bank (tag ps1);
  - softmax denominator rows come from ones columns in the packed V,
    written by per-tile GpSimd memsets;
  - input DMAs split across the SP and ACT hardware queues + GpSimd SW DGE;
  - tail reciprocals run as Ln->Exp(-x) on the (by then idle) ScalarE;
  - output is returned transposed; host un-transposes.
"""

import ml_dtypes
import numpy as np

import concourse.bass as bass
import concourse.tile as tile
from concourse import mybir
from concourse.bass_utils import run_bass_kernel_spmd

B, S, D, H, HD = 2, 2048, 128, 8, 16
QC = 512  # query rows per core
NCORES = 8
KT = 16  # number of 128-row key tiles
F32 = mybir.dt.float32
BF16 = mybir.dt.bfloat16
AF = mybir.ActivationFunctionType
OP = mybir.AluOpType
NPBF = ml_dtypes.bfloat16


# ---------------------------------------------------------------- host packing
def _head_cols(h):
    return [j * H + h for j in range(HD)]


def _fuse_qk(wq, wk):
    """[128, H*128] bf16: col block h = M_h^T = 0.25 * wq_h @ wk_h^T."""
    out = np.empty((D, H * D), np.float32)
    for h in range(H):
        c = _head_cols(h)
        out[:, D * h : D * (h + 1)] = 0.25 * (wq[:, c] @ wk[:, c].T)
    return out


def _pack32_cols(w, grp):
    """[D, 128]: col 32g+j (j<16) = w[:, j*H + (4*grp+g)], else 0."""
    out = np.zeros((D, 128), np.float32)
    for g in range(4):
        h = 4 * grp + g
        for j in range(HD):
            out[:, 32 * g + j] = w[:, j * H + h]
    return out


def _pack_w12(w, grp):
    """lhsT for the merge projection: row 32c+j = w[j*H + (4*grp+c), :]."""
    out = np.zeros((D, D), np.float32)
    for c in range(4):
        h = 4 * grp + c
        for j in range(HD):
            out[32 * c + j, :] = w[j * H + h, :]
    return out


def _sel_matrix():
    sel = np.zeros((128, 128), np.float32)
    for m in range(128):
        sel[32 * (m // 32) + 16, m] = 1.0
    return sel


def _split_multiwaits(nc):
    """Post-pass for walrus builds that accept only ONE sync-wait per
    instruction: split every instruction carrying N>1 waits into (N-1)
    single-wait NOPs on the same engine placed immediately before it."""
    uid = 0
    for f in nc.m.functions:
        for bb in f.blocks:
            il = bb.instructions
            if not any(
                i.sync_info is not None
                and i.sync_info.on_wait
                and len(i.sync_info.on_wait) > 1
                for i in il
            ):
                continue
            out = []
            for inst in il:
                si = inst.sync_info
                if si is not None and si.on_wait and len(si.on_wait) > 1:
                    waits = list(si.on_wait)
                    for w in waits[:-1]:
                        uid += 1
                        nop = mybir.InstNoOp(
                            name=f"WSPLIT-{uid}",
                            engine=inst.engine,
                            ins=[],
                            outs=[],
                            sync_info=mybir.SyncInfo(on_wait=[w], on_update=[]),
                        )
                        out.append(nop)
                    inst.sync_info = mybir.SyncInfo(
                        on_wait=[waits[-1]], on_update=list(si.on_update)
                    )
                out.append(inst)
            bb.instructions = out
    return nc


# ---------------------------------------------------------------- device build
def build_nc():
    nc = bass.Bass()

    def din(name, shape, dt=BF16):
        return nc.dram_tensor(name, list(shape), dt, kind="ExternalInput")

    xbT = din("xbT", (128, 2048))  # batch x_tgt transposed [chan, key]
    xqT = din("xqT", (128, 512))  # query slice of x_tgt, transposed
    eoT = din("eoT", (128, 512))  # query slice of enc_out, transposed
    xqF = din("xqF", (128, 512), F32)  # fp32 copy for the residual
    mgs = din("mgs", (128, 1024))  # self-attn fused M_h^T blocks
    mgc = din("mgc", (128, 1024))  # cross-attn fused M_h^T blocks
    wv_st = din("wv_st", (D, 512))  # [v_selfA | v_selfB | v_crossA | v_crossB]
    w1p = [din(f"w1p{g}", (D, D)) for g in range(2)]
    w2p = [din(f"w2p{g}", (D, D)) for g in range(2)]
    w3 = din("w3", (D, 512))
    w4r = din("w4r", (128, 512))  # col block j = w4[128j:128j+128, :]
    selt = din("selt", (128, 128), F32)  # SEL[p, m] = (p == 32*(m//32)+16)
    onec_f = din("onec_f", (128, 1), F32)  # fp32 ones column (LN sums)
    oner_b = din("oner_b", (1, 128))  # bf16 ones row (LN b broadcast)
    c127_b = din("c127_b", (1, 128))  # bf16 127s row (LN a broadcast)
    b1t = din("b1t", (128, 1), F32)
    b2t = din("b2t", (128, 1), F32)
    b3t = din("b3t", (128, 4), F32)
    b4t = din("b4t", (128, 1), F32)
    y = nc.dram_tensor("y", [128, 512], F32, kind="ExternalOutput")

    with tile.TileContext(nc) as tc:
        with tc.tile_pool(name="persist", bufs=1) as pp, \
             tc.tile_pool(name="pattn", bufs=1, space="PSUM") as pa, \
             tc.tile_pool(name="ebp", bufs=3) as ebp:

            def sbuf(name, shape, dt=F32):
                return pp.tile(list(shape), dt, name=name, tag=name)

            def load(name, dram, shape, dt=BF16, eng=None):
                t = sbuf(name, shape, dt)
                (eng or nc.sync).dma_start(out=t[:], in_=dram[:])
                return t

            # critical loads on the SP hardware queue, ordered by first use
            mgs_t = load("mgs", mgs, (128, 1024))
            xqT_t = load("xqT", xqT, (128, 512))
            xbT_t = load("xbT", xbT, (128, 2048))
            wv_t = load("wv", wv_st, (D, 512))
            mgc_t = load("mgc", mgc, (128, 1024))
            eoT_t = load("eoT", eoT, (128, 512))
            # needed by norm0/r1: ACT hardware queue (idle at t=0)
            sel_t = load("sel", selt, (128, 128), F32, nc.scalar)
            w1p_t = [load(f"w1p{g}", w1p[g], (D, D), BF16, nc.scalar)
                     for g in range(2)]
            xqF_t = load("xqF", xqF, (128, 512), F32, nc.scalar)
            b1_t = load("b1", b1t, (128, 1), F32, nc.scalar)
            b2_t = load("b2", b2t, (128, 1), F32, nc.scalar)

            v_all = sbuf("v_all", (128, KT, 512), BF16)
            g_s = [sbuf(f"gs{h}", (128, 512), BF16) for h in range(H)]
            g_c = [sbuf(f"gc{h}", (128, 512), BF16) for h in range(H)]

            # ---- setup work items, woven into the attention pipeline.
            # All share one PSUM bank (tag ps1) serialized with their casts.
            def g_item(msrc, xsrc, dst, name):
                gp = pa.tile([128, 512], F32, name=name, tag="ps1")
                nc.tensor.matmul(gp[:], lhsT=msrc, rhs=xsrc,
                                 start=True, stop=True)
                nc.vector.tensor_copy(out=dst, in_=gp[:])

            def v_item(t):
                vp = pa.tile([128, 512], F32, name=f"vp{t}", tag="ps1")
                nc.tensor.matmul(vp[:],
                                 lhsT=xbT_t[:, 128 * t : 128 * (t + 1)],
                                 rhs=wv_t[:], start=True, stop=True)
                nc.vector.tensor_copy(out=v_all[:, t, :], in_=vp[:])
                # softmax-denominator ones columns (col 16 of each 32-group)
                nc.gpsimd.memset(
                    v_all[:, t, :].rearrange("p (c x) -> p c x", x=32)[:, :, 16:17],
                    1.0,
                )

            def gs_item(h):
                return lambda: g_item(mgs_t[:, 128 * h : 128 * (h + 1)],
                                      xqT_t[:], g_s[h][:], f"gps{h}")

            def gc_item(h):
                return lambda: g_item(mgc_t[:, 128 * h : 128 * (h + 1)],
                                      eoT_t[:], g_c[h][:], f"gpc{h}")

            # G for the first set's two heads up front; everything else woven
            gs_item(0)()
            gs_item(1)()

            packed = {}  # (ai, grp) -> SBUF f32 accumulator
            rp = {}  # ai -> PSUM merge accumulator

            def attn_set(ai, st, g_heads, weave=()):
                """2 heads x 16 k-tiles, software-pipelined: scores(t) and
                exp(t) issue before PV(t-1) so the ScalarE exp stream never
                waits on PE; one optional setup item woven per tile."""
                h0 = 2 * st
                pv = [pa.tile([32, 512], F32, name=f"pv{ai}{st}{i}",
                              tag=f"pv{i}") for i in range(2)]
                ebs = [None] * KT
                for t in range(KT + 1):
                    if t < KT:
                        sc = pa.tile([128, 1024], F32, bufs=2,
                                     name=f"sc{ai}{st}{t}", tag="sc")
                        for i in range(2):
                            nc.tensor.matmul(
                                sc[:, 512 * i : 512 * (i + 1)],
                                lhsT=xbT_t[:, 128 * t : 128 * (t + 1)],
                                rhs=g_heads[h0 + i][:],
                                start=True, stop=True,
                            )
                        eb = ebp.tile([128, 1024], BF16, name="eb", tag="eb")
                        nc.scalar.activation(eb[:], sc[:], AF.Exp)
                        ebs[t] = eb
                        if t < len(weave) and weave[t] is not None:
                            weave[t]()
                    if t > 0:
                        tp = t - 1
                        for i in range(2):
                            h = h0 + i
                            v0 = 256 * ai + 128 * (h // 4) + 32 * (h % 4)
                            nc.tensor.matmul(
                                pv[i][:],
                                lhsT=v_all[:, tp, v0 : v0 + 32],
                                rhs=ebs[tp][:, 512 * i : 512 * (i + 1)],
                                start=(tp == 0), stop=(tp == KT - 1),
                                skip_group_check=True,
                            )
                for i in range(2):
                    h = h0 + i
                    grp = h // 4
                    if (ai, grp) not in packed:
                        packed[(ai, grp)] = pp.tile(
                            [128, 512], F32, name=f"acc{ai}{grp}",
                            tag=f"acc{ai}{grp}")
                    nc.vector.tensor_copy(
                        out=packed[(ai, grp)][32 * (h % 4) : 32 * (h % 4) + 32, :],
                        in_=pv[i][:],
                    )

            def norm_grp(ai, grp, wp_t, recip_on_act=False):
                """softmax-normalize one 4-head group and accumulate its
                merge projection into rp[ai]."""
                acc = packed[(ai, grp)]
                sbc = pa.tile([128, 512], F32, name=f"sbc{ai}{grp}",
                              tag="ps1")
                nc.tensor.matmul(sbc[:], lhsT=sel_t[:], rhs=acc[:],
                                 start=True, stop=True)
                rb = pp.tile([128, 512], F32, name=f"rb{ai}{grp}", tag="rb")
                if recip_on_act:
                    # 1/x as exp(-ln x) on the (idle, post-exp) ScalarE:
                    # ~2x lower latency than the DVE iterative divide.
                    lnr = pp.tile([128, 512], F32, name=f"lnr{ai}{grp}",
                                  tag="lnr")
                    nc.scalar.activation(lnr[:], sbc[:], AF.Ln)
                    nc.scalar.activation(rb[:], lnr[:], AF.Exp, scale=-1.0)
                else:
                    nc.vector.reciprocal(out=rb[:], in_=sbc[:])
                sn = pp.tile([128, 512], BF16, name=f"sn{ai}{grp}",
                             tag=f"sn{grp}")
                nc.vector.tensor_mul(sn[:], acc[:], rb[:])
                if ai not in rp:
                    rp[ai] = pa.tile([128, 512], F32, name=f"rp{ai}",
                                     tag="ps0")
                nc.tensor.matmul(rp[ai][:], lhsT=wp_t[grp][:], rhs=sn[:],
                                 start=(grp == 0), stop=(grp == 1),
                                 skip_group_check=True)

            # self-attention; V and remaining G matmuls woven into sets 0-1
            set0_weave = [
                (lambda t=t: (v_item(t), gs_item(t + 2)()) if t < 6
                 else v_item(t)) for t in range(KT)
            ]
            set1_weave = [gc_item(h) for h in range(H)]
            attn_set(0, 0, g_s, set0_weave)
            attn_set(0, 1, g_s, set1_weave)
            norm_grp(0, 0, w1p_t)
            attn_set(0, 2, g_s)
            attn_set(0, 3, g_s)
            norm_grp(0, 1, w1p_t)
            r1T = sbuf("r1T", (128, 512))
            # r1 = (sa@w1 + b1) + x_tgt
            nc.vector.scalar_tensor_tensor(
                out=r1T[:], in0=rp[0][:], scalar=b1_t[:], in1=xqF_t[:],
                op0=OP.add, op1=OP.add)

            # remaining weights on the GpSimd SW-DGE queue (needed from
            # norm1 / tail onwards)
            w2p_t = [load(f"w2p{g}", w2p[g], (D, D), BF16, nc.gpsimd)
                     for g in range(2)]
            w3_t = load("w3", w3, (D, 512), BF16, nc.gpsimd)
            w4_t = load("w4", w4r, (128, 512), BF16, nc.gpsimd)
            onec_t = load("onec", onec_f, (128, 1), F32, nc.gpsimd)
            oner_t = load("oner", oner_b, (1, 128), BF16, nc.gpsimd)
            c127_t = load("c127", c127_b, (1, 128), BF16, nc.gpsimd)
            b3_t = load("b3", b3t, (128, 4), F32, nc.gpsimd)
            b4_t = load("b4", b4t, (128, 1), F32, nc.gpsimd)

            # cross-attention
            attn_set(1, 0, g_c)
            attn_set(1, 1, g_c)
            norm_grp(1, 0, w2p_t)
            attn_set(1, 2, g_c)
            attn_set(1, 3, g_c)
            norm_grp(1, 1, w2p_t, recip_on_act=True)
            r2T = sbuf("r2T", (128, 512))
            nc.vector.scalar_tensor_tensor(
                out=r2T[:], in0=rp[1][:], scalar=b2_t[:], in1=r1T[:],
                op0=OP.add, op1=OP.add)

            # ---------------- layernorm (x - m) / var, var unbiased
            sqf = sbuf("sqf", (128, 512))
            nc.vector.tensor_mul(sqf[:], r2T[:], r2T[:])
            spm = pa.tile([1, 512], F32, name="spm", tag="ps1")
            nc.tensor.matmul(spm[:], lhsT=onec_t[:], rhs=r2T[:],
                             start=True, stop=True)
            sps = pa.tile([1, 512], F32, name="sps", tag="ps0")
            nc.tensor.matmul(sps[:], lhsT=onec_t[:], rhs=sqf[:],
                             start=True, stop=True)
            msb = sbuf("msb", (1, 512))
            nc.vector.tensor_copy(out=msb[:], in_=spm[:])
            # sum((x-m)^2) = ss - m^2/128  (m here = column sum)
            tm = sbuf("tm", (1, 512))
            nc.vector.scalar_tensor_tensor(
                out=tm[:], in0=msb[:], scalar=-1.0 / 128, in1=msb[:],
                op0=OP.mult, op1=OP.mult)
            tv = sbuf("tv", (1, 512))
            nc.vector.tensor_add(tv[:], sps[:], tm[:])
            # ra = 1/sum((x-m)^2); the 127 of the unbiased var is folded into
            # the broadcast lhsT (c127). 1/x via Ln->Exp on idle ScalarE.
            lnv = sbuf("lnv", (1, 512))
            nc.scalar.activation(lnv[:], tv[:], AF.Ln)
            ra = sbuf("ra", (1, 512))
            nc.scalar.activation(ra[:], lnv[:], AF.Exp, scale=-1.0)
            # b = -m/128 * 127 * ra  (m = sum/128 folded in)
            rb2 = sbuf("rb2", (1, 512))
            nc.vector.scalar_tensor_tensor(
                out=rb2[:], in0=ra[:], scalar=-127.0 / 128, in1=msb[:],
                op0=OP.mult, op1=OP.mult)
            ab_b = sbuf("ab_b", (1, 1024), BF16)
            nc.vector.tensor_copy(out=ab_b[:, :512], in_=ra[:])
            nc.vector.tensor_copy(out=ab_b[:, 512:], in_=rb2[:])
            abc = pa.tile([128, 1024], F32, name="abc", tag="sc", bufs=2)
            nc.tensor.matmul(abc[:, :512], lhsT=c127_t[:], rhs=ab_b[:, :512],
                             start=True, stop=True)
            nc.tensor.matmul(abc[:, 512:], lhsT=oner_t[:], rhs=ab_b[:, 512:],
                             start=True, stop=True)
            lnf = sbuf("lnf", (128, 512))
            nc.vector.tensor_mul(lnf[:], r2T[:], abc[:, :512])
            lnT = sbuf("lnT", (128, 512), BF16)
            nc.vector.tensor_add(lnT[:], lnf[:], abc[:, 512:])

            # ---------------- FFN
            h_sb = []
            for j in range(4):
                hp = pa.tile([128, 512], F32, name=f"hp{j}",
                             tag=f"pv{j % 2}")
                nc.tensor.matmul(hp[:],
                                 lhsT=w3_t[:, 128 * j : 128 * (j + 1)],
                                 rhs=lnT[:], start=True, stop=True)
                hs = sbuf(f"hs{j}", (128, 512), BF16)
                nc.vector.tensor_scalar(
                    out=hs[:], in0=hp[:], scalar1=b3_t[:, j : j + 1],
                    scalar2=0.0, op0=OP.add, op1=OP.max,
                )
                h_sb.append(hs)
            op_ = pa.tile([128, 512], F32, name="op", tag="ps0")
            for j in range(4):
                nc.tensor.matmul(op_[:],
                                 lhsT=w4_t[:, 128 * j : 128 * (j + 1)],
                                 rhs=h_sb[j][:],
                                 start=(j == 0), stop=(j == 3),
                                 skip_group_check=True)
            oT = sbuf("oT", (128, 512))
            nc.vector.scalar_tensor_tensor(
                out=oT[:], in0=op_[:], scalar=b4_t[:], in1=r2T[:],
                op0=OP.add, op1=OP.add)
            nc.sync.dma_start(out=y[:], in_=oT[:])

    return nc


_CACHED = {}


def _get_nc():
    if "nc" not in _CACHED:
        _CACHED["nc"] = _split_multiwaits(build_nc())
    return _CACHED["nc"]


def _host_inputs(x_tgt, enc_out, self_wq, self_wk, self_wv, cross_wq, cross_wk,
                 cross_wv, w1, b1, w2, b2, w3, b3, w4, b4):
    bf = lambda a: np.ascontiguousarray(a, dtype=NPBF)
    f32 = lambda a: np.ascontiguousarray(a, dtype=np.float32)
    shared = {
        "mgs": bf(_fuse_qk(self_wq, self_wk)),
        "mgc": bf(_fuse_qk(cross_wq, cross_wk)),
        "wv_st": bf(np.concatenate(
            [_pack32_cols(self_wv, 0), _pack32_cols(self_wv, 1),
             _pack32_cols(cross_wv, 0), _pack32_cols(cross_wv, 1)], axis=1
        )),
        "w1p0": bf(_pack_w12(w1, 0)), "w1p1": bf(_pack_w12(w1, 1)),
        "w2p0": bf(_pack_w12(w2, 0)), "w2p1": bf(_pack_w12(w2, 1)),
        "w3": bf(w3),
        "w4r": bf(w4.reshape(4, 128, 128).transpose(1, 0, 2).reshape(128, 512)),
        "selt": f32(_sel_matrix()),
        "onec_f": f32(np.ones((128, 1), np.float32)),
        "oner_b": bf(np.ones((1, 128), np.float32)),
        "c127_b": bf(np.full((1, 128), 127.0, np.float32)),
        "b1t": f32(b1.reshape(128, 1)),
        "b2t": f32(b2.reshape(128, 1)),
        "b3t": f32(b3.reshape(4, 128).T),
        "b4t": f32(b4.reshape(128, 1)),
    }
    in_maps = []
    for c in range(NCORES):
        b, qb = divmod(c, 4)
        q0 = qb * QC
        im = dict(shared)
        im["xbT"] = bf(x_tgt[b].T)
        im["xqT"] = bf(x_tgt[b, q0 : q0 + QC].T)
        im["xqF"] = f32(x_tgt[b, q0 : q0 + QC].T)
        im["eoT"] = bf(enc_out[b, q0 : q0 + QC].T)
        in_maps.append(im)
    return in_maps


def _unshuf(y):
    """[128, 512] transposed output -> [512, 128] natural rows."""
    return np.ascontiguousarray(y.T)


def run_on_device(in_maps, **kw):
    nc = _get_nc()
    return run_bass_kernel_spmd(nc, in_maps, list(range(NCORES)), **kw)


def kernel(x_tgt, enc_out, self_wq, self_wk, self_wv, cross_wq, cross_wk,
           cross_wv, w1, b1, w2, b2, w3, b3, w4, b4, mask_src=None,
           mask_tgt=None, **_unused):
    args = [x_tgt, enc_out, self_wq, self_wk, self_wv, cross_wq, cross_wk,
            cross_wv, w1, b1, w2, b2, w3, b3, w4, b4]
    args = [np.asarray(a, dtype=np.float32) for a in args]
    in_maps = _host_inputs(*args)
    res = run_on_device(in_maps)
    out = np.empty((B, S, D), np.float32)
    for c in range(NCORES):
        b, qb = divmod(c, 4)
        out[b, qb * QC : (qb + 1) * QC] = _unshuf(res.results[c]["y"])
    return out
